# revision 11
# baseline (speedup 1.0000x reference)
"""Leaky integrator y_t = beta*y_{t-1} + x_t over (T=2048, B=32, D=1024) f32.

Strategy: the scan is independent per (b, d) lane, so shard the flattened
B*D = 32768 lanes across 8 NeuronCores (4096 lanes each). On each core the
recurrence is computed as blocked matmuls with decay matrices: for time
blocks of P=128,

    y[k] = sum_{m=0..3} W_m @ x[k-m],   W_m[i, j] = beta^(i - j + m*P)
    (W_0 lower-triangular; history beyond 4 blocks is < beta^512 ~ 4e-12,
     far below fp32 resolution of the reference itself)

All fp32 on the PE; PSUM accumulates the 4 contributions; DVE copies
PSUM->SBUF; HWDGE DMAs stream whole 128-row time blocks (contiguous 2 MiB)
in and out. Memory-bound: 64 MiB of HBM traffic per core.
"""

import numpy as np

import concourse.bass as bass
import concourse.bacc as bacc
import concourse.mybir as mybir
import concourse.tile as tile
from concourse.bass_utils import run_bass_kernel_spmd

BETA = 0.95
T, B, D = 2048, 32, 1024
N_CORES = 8
E = B * D                  # 32768 independent scan lanes
E_LOC = E // N_CORES       # 4096 lanes per core
P = 128                    # time-block size == SBUF partitions
NB = T // P                # 16 time blocks
SW = 512                   # strip width (fp32 moving-operand / PSUM-bank max)
NS = E_LOC // SW           # 8 strips per core
M_WIN = 4                  # history window in blocks


def _weights() -> np.ndarray:
    # lhsT layout, all windows side by side: w[:, m*P:(m+1)*P][j, i]
    #   = W_m[i, j] = beta^(i - j + m*P)
    i = np.arange(P)[None, :].astype(np.int64)
    j = np.arange(P)[:, None].astype(np.int64)
    mats = []
    for m in range(M_WIN):
        e = i - j + m * P
        mats.append(np.where(e >= 0, np.float64(BETA) ** e, 0.0))
    return np.concatenate(mats, axis=1).astype(np.float32)  # (P, M_WIN*P)


def _build_program(reps: int = 1) -> bass.Bass:
    """reps>1 repeats the whole pipeline (same DRAM in/out) — used only for
    slope-based HW timing; the math is idempotent."""
    f32 = mybir.dt.float32
    nc = bacc.Bacc()
    x_in = nc.declare_dram_parameter("x", [T, E_LOC], f32, isOutput=False)
    w_in = nc.declare_dram_parameter("w", [P, M_WIN * P], f32, isOutput=False)
    y_out = nc.declare_dram_parameter("y", [T, E_LOC], f32, isOutput=True)

    with tile.TileContext(nc) as tc:
        with (
            tc.tile_pool(name="wpool", bufs=1) as wpool,
            tc.tile_pool(name="xpool", bufs=6) as xpool,
            tc.tile_pool(name="ypool", bufs=3) as ypool,
            tc.tile_pool(name="psum", bufs=8, space="PSUM") as psum,
        ):
            w_all = wpool.tile([P, M_WIN * P], f32)
            nc.sync.dma_start(w_all[:], w_in[:])

            for _rep in range(reps):
                xhist = {}
                for k in range(NB):
                    xt = xpool.tile([P, E_LOC], f32, tag="xt")
                    nc.sync.dma_start(xt[:], x_in[k * P:(k + 1) * P, :])
                    xhist[k] = xt

                    yt = ypool.tile([P, E_LOC], f32, tag="yt")
                    # m=0 (fresh x_k) goes LAST so the group's first matmul
                    # (old x tiles) carries the psum-release wait and the
                    # m=0 matmul carries only the x-DMA wait.
                    ms = [m for m in (1, 2, 3) if k - m >= 0] + [0]
                    for s in range(NS):
                        ps = psum.tile([P, SW], f32, tag="ps")
                        for q, m in enumerate(ms):
                            nc.tensor.matmul(
                                ps[:],
                                w_all[:, m * P:(m + 1) * P],
                                xhist[k - m][:, s * SW:(s + 1) * SW],
                                start=(q == 0),
                                stop=(q == len(ms) - 1),
                            )
                        nc.vector.tensor_copy(yt[:, s * SW:(s + 1) * SW], ps[:])

                    nc.sync.dma_start(y_out[k * P:(k + 1) * P, :], yt[:])
    return nc


def make_in_maps(x: np.ndarray):
    x = np.ascontiguousarray(x, dtype=np.float32)
    assert x.shape == (T, B, D), x.shape
    xf = x.reshape(T, E)
    w = _weights()
    return [
        {"x": np.ascontiguousarray(xf[:, c * E_LOC:(c + 1) * E_LOC]), "w": w}
        for c in range(N_CORES)
    ]


def run_full(x: np.ndarray, trace: bool = False, **spmd_kwargs):
    """Run on all 8 cores; returns (y_full, BassKernelResults)."""
    in_maps = make_in_maps(x)
    nc = _build_program()
    if not nc.is_finalized():
        nc.finalize()  # Bacc: runs wait-splitting + register allocation
    res = run_bass_kernel_spmd(
        nc, in_maps, list(range(N_CORES)), trace=trace, **spmd_kwargs
    )
    y = np.concatenate([res.results[c]["y"] for c in range(N_CORES)], axis=1)
    return y.reshape(T, B, D), res


def kernel(x: np.ndarray) -> np.ndarray:
    y, _ = run_full(x, trace=False)
    return y
